# revision 36
# baseline (speedup 1.0000x reference)
"""CARAFE content-aware upsampling kernel for Trainium2 (Bass/Tile), 8 NeuronCores.

Problem (hardcoded): features [4, 256, 64, 64] f32, masks [4, 25, 128, 128] f32,
K=5, G=1, S=2 -> output [4, 256, 128, 128] f32.

Strategy
--------
Sharding: 8 cores = (batch n in 0..3) x (output-row half yh in 0..1); each core
computes out[n, :, yh*64:(yh+1)*64, :] for ALL 256 channels.

Compute mapping: each output block of (4 row-pairs x 16 columns) = 128 output
positions depends on an 8-row x 12-col window of the padded input feature map.
That window is contracted in TWO PSUM-accumulated matmuls, one per 4-row
feature half-window, k = wpw*4 + hpr (48 rows; 40 for the edge col-blocks
whose outer wpw hit zero feature padding):

  out[c, pos] = sum_half sum_k ftr_half[k, c] * bnd_half[k, pos]

The 25 CARAFE taps split as kr = (4*half + hpr) - p4, dw = wpw - xl//2 (host
bakes this into the banded bnd operand; entries outside [0,5) are zero).
Splitting by half-window means each 4-row feature slice is stored ONCE and
shared by the two adjacent row-groups that read it (mm2 of hgrp j-1 and mm1 of
hgrp j use the same stationary), eliminating the vertical replication of the
feature operand. 256 matmuls x 128 moving cols per core (~33k PE cycles,
LoadStationary is double-buffered against the moving pass), PSUM output lands
directly in [c, y-major] layout.

Dataflow: feature halves and banded masks are packed per row-group chunk into
two DRAM tensors (48-row mid blocks, 40-row edge blocks) and streamed with 10
DMAs; per row-group a [128, 2048] PSUM tile (4 banks) collects 16 two-matmul
accumulation groups (disjoint 128-col slices); DVE and ACT each cast one
channel-half to bf16 into a shared staging tile (reordering to y-major), and
one DMA per row-group writes [256ch x 8row x 128col] to DRAM. The host
upcasts to f32. Total DMA: ~3.05 MiB in + 4 MiB out per core.
"""

import sys

sys.path.insert(0, "/opt/trn_rl_repo")

import numpy as np
import ml_dtypes

import concourse.bacc as bacc
import concourse.mybir as mybir
from concourse import tile
from concourse import bass_utils

N, C, H, W = 4, 256, 64, 64
KK = 5
HO, WO = 128, 128
NCORES = 8

HPL = 36          # padded input rows per core (32 pairs + 4 tap overlap)
WP = 68           # padded input cols
NHG = 8           # row-groups per core (4 row-pairs each)
NXB = 8           # col-blocks per core (16 output cols each)
NJ = 9            # feature half-windows (4 rows each)
KH = 48           # contraction rows per half: 12 wpw x 4 hpr
KHE = 40          # edge col-blocks: 10 wpw x 4 hpr
CHUNKS = (1, 1, 2, 2, 2)   # hgrps per input DMA chunk

BF16 = ml_dtypes.bfloat16

# halves shipped per chunk: chunk0 ships j in [0, g+1), chunk c ships
# (h0+1 .. h0+g]; half j is first needed by hgrp j-1 (mm2) / hgrp j (mm1)
_half_loc = {}
_chunk_halves = []
_h0s = []
h0 = 0
for _ci, _g in enumerate(CHUNKS):
    js = list(range(0, _g + 1)) if _ci == 0 else list(range(h0 + 1, h0 + _g + 1))
    for _jl, _j in enumerate(js):
        _half_loc[_j] = (_ci, _jl)
    _chunk_halves.append(js)
    _h0s.append(h0)
    h0 += _g
_chunk_of_hgrp = [next(c for c in range(len(CHUNKS))
                       if _h0s[c] <= h < _h0s[c] + CHUNKS[c]) for h in range(NHG)]

# per-chunk column widths: mid [48 rows] and edge [40 rows], both half-shared
_MW = [len(_chunk_halves[c]) * 6 * C + 2 * CHUNKS[c] * 6 * 128
       for c in range(len(CHUNKS))]
_EW = [len(_chunk_halves[c]) * 2 * C + 2 * CHUNKS[c] * 2 * 128
       for c in range(len(CHUNKS))]
MTOT, ETOT = sum(_MW), sum(_EW)


def _host_prep(features: np.ndarray, masks: np.ndarray):
    """Per-core packed chunk operands fbm [48, MTOT], fbe [40, ETOT] (bf16)."""
    fbms, fbes = [], []
    for i in range(NCORES):
        n, yh = divmod(i, 2)
        feat_pad = np.zeros((HPL, WP, C), np.float32)
        featT = features[n].transpose(1, 2, 0)  # [H, W, C]
        r0 = yh * 32 - 2
        lo, hi = max(0, -r0), min(HPL, H - r0)
        feat_pad[lo:hi, 2:2 + W, :] = featT[r0 + lo:r0 + hi]

        # bnd6[wpw, hpw, hgrp, xblk, pos]
        ml = masks[n, :, yh * 64:(yh + 1) * 64, :]
        bnd6 = np.zeros((12, 8, NHG, NXB, 4, 2, 16), np.float32)
        s = bnd6.strides
        for kr in range(KK):
            for dw in range(KK):
                # dest dims (p4, hgrp, xblk, xw, py, q):
                #   bnd6[dw+xw, p4+kr, hgrp, xblk, p4, py, 2*xw+q]
                dv = np.lib.stride_tricks.as_strided(
                    bnd6[dw, kr],
                    shape=(4, NHG, NXB, 8, 2, 2),
                    strides=(s[1] + s[4], s[2], s[3], s[0] + 2 * s[6], s[5], s[6]),
                )
                sv = ml[kr * KK + dw].reshape(NHG, 4, 2, NXB, 8, 2)
                dv[...] = sv.transpose(1, 0, 3, 4, 2, 5)
        bnd6 = bnd6.reshape(12, 8, NHG, NXB, 128)

        def ftr_half(j, xblk):
            if xblk == 0:
                wps = np.arange(2, 12)      # wp = wpw; wpw 0,1 are pad
            elif xblk == 7:
                wps = np.arange(56, 66)     # wp = 56+wpw; wpw 10,11 are pad
            else:
                wps = 8 * xblk + np.arange(12)
            blk = feat_pad[4 * j:4 * j + 4][:, wps]         # [4 hpr, nw, C]
            return blk.transpose(1, 0, 2).reshape(-1, C)    # k = w*4 + hpr

        def bnd_half(half, hgrp, xblk):
            hp = slice(4 * half, 4 * half + 4)
            if xblk == 0:
                b = bnd6[2:, hp, hgrp, 0]
            elif xblk == 7:
                b = bnd6[:10, hp, hgrp, 7]
            else:
                b = bnd6[:, hp, hgrp, xblk]
            return b.reshape(-1, 128)                       # k = wpw*4 + hpr

        def cat(pieces):
            return np.concatenate(
                [x.transpose(1, 0, 2).reshape(x.shape[1], -1) for x in pieces],
                axis=1)

        fbm_segs, fbe_segs = [], []
        for ci, g in enumerate(CHUNKS):
            h0, js = _h0s[ci], _chunk_halves[ci]
            hs = range(h0, h0 + g)
            if ci == 0:
                # chunk0 mid is split: xblk 1-3 pieces first (small leading
                # DMA lets PE start early), then xblk 4-6
                fbm_segs.append(cat(
                    [np.stack([ftr_half(j, xb) for xb in range(1, 4)]) for j in js]
                    + [np.stack([bnd_half(0, 0, xb) for xb in range(1, 4)])]
                    + [np.stack([bnd_half(1, 0, xb) for xb in range(1, 4)])]
                    + [np.stack([ftr_half(j, xb) for xb in range(4, 7)]) for j in js]
                    + [np.stack([bnd_half(0, 0, xb) for xb in range(4, 7)])]
                    + [np.stack([bnd_half(1, 0, xb) for xb in range(4, 7)])]))
            else:
                fbm_segs.append(cat(
                    [np.stack([ftr_half(j, xb) for xb in range(1, 7)]) for j in js]
                    + [np.stack([bnd_half(0, h, xb) for xb in range(1, 7)]) for h in hs]
                    + [np.stack([bnd_half(1, h, xb) for xb in range(1, 7)]) for h in hs]))
            fbe_segs.append(cat(
                [np.stack([ftr_half(j, xb) for xb in (0, 7)]) for j in js]
                + [np.stack([bnd_half(0, h, xb) for xb in (0, 7)]) for h in hs]
                + [np.stack([bnd_half(1, h, xb) for xb in (0, 7)]) for h in hs]))
        fbms.append(np.concatenate(fbm_segs, axis=1).astype(BF16))
        fbes.append(np.concatenate(fbe_segs, axis=1).astype(BF16))
    return fbms, fbes


_NC_CACHE = []


def _build_nc():
    """Build + compile the single-core Tile program (same for all 8 cores)."""
    if _NC_CACHE:
        return _NC_CACHE[0]

    nc = bacc.Bacc("TRN2", target_bir_lowering=False, debug=False)
    fbm = nc.dram_tensor("fbm", [KH, MTOT], mybir.dt.bfloat16,
                         kind="ExternalInput").ap()
    fbe = nc.dram_tensor("fbe", [KHE, ETOT], mybir.dt.bfloat16,
                         kind="ExternalInput").ap()
    out = nc.dram_tensor("out", [C, 64 * 128], mybir.dt.bfloat16,
                         kind="ExternalOutput").ap()
    ov = out.rearrange("(ch c) (hgrp f) -> ch c hgrp f", ch=2, hgrp=NHG)

    with tile.TileContext(nc) as tc:
        with (
            tc.tile_pool(name="fbp", bufs=2 * len(CHUNKS)) as fbp,
            tc.tile_pool(name="stp", bufs=8) as stp,
            tc.tile_pool(name="pp", bufs=4, space="PSUM") as pp,
        ):
            # warm-up: ACT function table (so it doesn't gate the first real
            # copy) and the PE p-state ramp (~3.4us of dummy matmuls into a
            # scratch psum slice keeps PE continuously busy until real
            # operands arrive, so real matmuls start at full clock)
            dm = stp.tile([KH, 512], mybir.dt.bfloat16, name="dm", tag="dm")
            nc.vector.memset(dm[:], 0)
            nc.scalar.copy(dm[:1, 1:2], dm[:1, 0:1])
            dps = pp.tile([128, 1024], mybir.dt.float32, name="dps", tag="ps")
            for w in range(8):
                nc.tensor.matmul(dps[:, (w % 2) * 512:(w % 2) * 512 + 512],
                                 dm[:, 0:128], dm[:], start=True, stop=True)

            mtiles, etiles = [], []
            moff = eoff = 0
            for ci, g in enumerate(CHUNKS):
                tm = fbp.tile([KH, _MW[ci]], mybir.dt.bfloat16,
                              name="fbm", tag="fbm")
                te = fbp.tile([KHE, _EW[ci]], mybir.dt.bfloat16,
                              name="fbe", tag="fbe")
                if ci == 0:
                    # small leading DMA (xblk 1-3 operands) for early PE start
                    nc.sync.dma_start(tm[:, 0:2304], fbm[:, 0:2304])
                    nc.sync.dma_start(tm[:, 2304:4608], fbm[:, moff + 2304:moff + 4608])
                else:
                    nc.sync.dma_start(tm[:], fbm[:, moff:moff + _MW[ci]])
                nc.sync.dma_start(te[:], fbe[:, eoff:eoff + _EW[ci]])
                moff += _MW[ci]
                eoff += _EW[ci]
                mtiles.append(tm)
                etiles.append(te)

            for hgrp in range(NHG):
                ci = _chunk_of_hgrp[hgrp]
                g, nh, hloc = CHUNKS[ci], len(_chunk_halves[ci]), hgrp - _h0s[ci]
                st = stp.tile([128, 2048], mybir.dt.bfloat16, name="st", tag="st")
                dv = st.rearrange(
                    "c (ch p4 py xblk xl) -> c ch xblk p4 py xl",
                    ch=2, p4=4, py=2, xblk=8,
                )
                # two psum half-tiles (2 banks each) per hgrp for finer recycling;
                # edge col-blocks (gated by the fbe DMA) go last in each half
                for hf, xorder in ((0, (1, 2, 3, 0)), (1, (4, 5, 6, 7))):
                    ps = pp.tile([128, 1024], mybir.dt.float32, name="ps", tag="ps")
                    for xblk in xorder:
                        xl4 = xblk - 4 * hf
                        if xblk in (0, 7):
                            # edge col-block: half-shared 40-row matmul pair
                            e = 0 if xblk == 0 else 1
                            for ch in range(2):
                                od = ps[:, (xl4 * 2 + ch) * 128:
                                        (xl4 * 2 + ch + 1) * 128]
                                for half in range(2):
                                    cj, jl = _half_loc[hgrp + half]
                                    lhsT = etiles[cj][
                                        :, (jl * 2 + e) * C + ch * 128:
                                           (jl * 2 + e) * C + ch * 128 + 128]
                                    boff = nh * 2 * C + half * g * 2 * 128
                                    rhs = etiles[ci][
                                        :, boff + (hloc * 2 + e) * 128:
                                           boff + (hloc * 2 + e + 1) * 128]
                                    nc.tensor.matmul(od, lhsT, rhs,
                                                     start=(half == 0),
                                                     stop=(half == 1))
                            continue
                        xb = xblk - 1
                        for ch in range(2):
                            od = ps[:, (xl4 * 2 + ch) * 128:(xl4 * 2 + ch + 1) * 128]
                            for half in range(2):
                                cj, jl = _half_loc[hgrp + half]
                                if cj == 0:
                                    sub, xbs = divmod(xb, 3)
                                    fo = sub * 2304 + jl * 768 + xbs * 256
                                    lhsT = mtiles[0][:, fo + ch * 128:
                                                     fo + ch * 128 + 128]
                                else:
                                    lhsT = mtiles[cj][
                                        :, (jl * 6 + xb) * C + ch * 128:
                                           (jl * 6 + xb) * C + ch * 128 + 128]
                                if ci == 0:
                                    sub, xbs = divmod(xb, 3)
                                    bo = sub * 2304 + 1536 + half * 384 + xbs * 128
                                    rhs = mtiles[0][:, bo:bo + 128]
                                else:
                                    boff = nh * 6 * C + half * g * 6 * 128
                                    rhs = mtiles[ci][
                                        :, boff + (hloc * 6 + xb) * 128:
                                           boff + (hloc * 6 + xb + 1) * 128]
                                nc.tensor.matmul(od, lhsT, rhs,
                                                 start=(half == 0), stop=(half == 1))
                    # psum cols (xblk4, ch, p4, py, xl) -> staging (ch, p4, py, xblk, xl)
                    sv = ps.rearrange(
                        "c (xblk ch p4 py xl) -> c ch xblk p4 py xl",
                        xblk=4, ch=2, p4=4, py=2,
                    )
                    if hgrp == NHG - 1 and hf == 0:
                        # last hgrp: put both hf0 copies on DVE (it has slack)
                        # so hf1 drains on two fresh engines
                        nc.vector.tensor_copy(dv[:, 0, 0:4], sv[:, 0])
                        nc.vector.tensor_copy(dv[:, 1, 0:4], sv[:, 1])
                    else:
                        nc.vector.tensor_copy(dv[:, 0, 4 * hf:4 * hf + 4], sv[:, 0])
                        nc.scalar.copy(dv[:, 1, 4 * hf:4 * hf + 4], sv[:, 1])
                sov = st.rearrange("c (ch f) -> c ch f", ch=2)
                nc.sync.dma_start(ov[:, :, hgrp, :].rearrange("ch c f -> c ch f"), sov)

    nc.compile()
    _NC_CACHE.append(nc)
    return nc


def kernel(features: np.ndarray, masks: np.ndarray) -> np.ndarray:
    features = np.ascontiguousarray(features, dtype=np.float32)
    masks = np.ascontiguousarray(masks, dtype=np.float32)
    fbms, fbes = _host_prep(features, masks)

    nc = _build_nc()
    in_maps = [{"fbm": fbms[i], "fbe": fbes[i]} for i in range(NCORES)]

    res = bass_utils.run_bass_kernel_spmd(nc, in_maps, list(range(NCORES)))

    out = np.empty((N, C, HO, WO), np.float32)
    for i in range(NCORES):
        n, yh = divmod(i, 2)
        out[n, :, yh * 64:(yh + 1) * 64, :] = (
            res.results[i]["out"].astype(np.float32).reshape(C, 64, 128)
        )
    return out


# revision 38
# speedup vs baseline: 1.0483x; 1.0483x over previous
"""CARAFE content-aware upsampling kernel for Trainium2 (Bass/Tile), 8 NeuronCores.

Problem (hardcoded): features [4, 256, 64, 64] f32, masks [4, 25, 128, 128] f32,
K=5, G=1, S=2 -> output [4, 256, 128, 128] f32.

Strategy
--------
Sharding: 8 cores = (batch n in 0..3) x (output-row half yh in 0..1); each core
computes out[n, :, yh*64:(yh+1)*64, :] for ALL 256 channels.

Compute mapping: each output block of (4 row-pairs x 16 columns) = 128 output
positions depends on an 8-row x 12-col window of the padded input feature map.
That window is contracted in TWO PSUM-accumulated matmuls, one per 4-row
feature half-window, k = wpw*4 + hpr (48 rows; 40 for the edge col-blocks
whose outer wpw hit zero feature padding):

  out[c, pos] = sum_half sum_k ftr_half[k, c] * bnd_half[k, pos]

The 25 CARAFE taps split as kr = (4*half + hpr) - p4, dw = wpw - xl//2 (host
bakes this into the banded bnd operand; entries outside [0,5) are zero).
Splitting by half-window means each 4-row feature slice is stored ONCE and
shared by the two adjacent row-groups that read it (mm2 of hgrp j-1 and mm1 of
hgrp j use the same stationary), eliminating the vertical replication of the
feature operand. 256 matmuls x 128 moving cols per core (~33k PE cycles,
LoadStationary is double-buffered against the moving pass), PSUM output lands
directly in [c, y-major] layout.

Dataflow: feature halves and banded masks are packed per row-group chunk into
two DRAM tensors (48-row mid blocks, 40-row edge blocks) and streamed with 10
DMAs; per row-group a [128, 2048] PSUM tile (4 banks) collects 16 two-matmul
accumulation groups (disjoint 128-col slices); DVE and ACT each cast one
channel-half to bf16 into a shared staging tile (reordering to y-major), and
one DMA per row-group writes [256ch x 8row x 128col] to DRAM. The host
upcasts to f32. Total DMA: ~3.05 MiB in + 4 MiB out per core.
"""

import sys

sys.path.insert(0, "/opt/trn_rl_repo")

import numpy as np
import ml_dtypes

import concourse.bacc as bacc
import concourse.mybir as mybir
from concourse import tile
from concourse import bass_utils

N, C, H, W = 4, 256, 64, 64
KK = 5
HO, WO = 128, 128
NCORES = 8

HPL = 36          # padded input rows per core (32 pairs + 4 tap overlap)
WP = 68           # padded input cols
NHG = 8           # row-groups per core (4 row-pairs each)
NXB = 8           # col-blocks per core (16 output cols each)
NJ = 9            # feature half-windows (4 rows each)
KH = 48           # contraction rows per half: 12 wpw x 4 hpr
KHE = 40          # edge col-blocks: 10 wpw x 4 hpr
CHUNKS = (1, 1, 2, 2, 2)   # hgrps per input DMA chunk

BF16 = ml_dtypes.bfloat16

# halves shipped per chunk: chunk0 ships j in [0, g+1), chunk c ships
# (h0+1 .. h0+g]; half j is first needed by hgrp j-1 (mm2) / hgrp j (mm1)
_half_loc = {}
_chunk_halves = []
_h0s = []
h0 = 0
for _ci, _g in enumerate(CHUNKS):
    js = list(range(0, _g + 1)) if _ci == 0 else list(range(h0 + 1, h0 + _g + 1))
    for _jl, _j in enumerate(js):
        _half_loc[_j] = (_ci, _jl)
    _chunk_halves.append(js)
    _h0s.append(h0)
    h0 += _g
_chunk_of_hgrp = [next(c for c in range(len(CHUNKS))
                       if _h0s[c] <= h < _h0s[c] + CHUNKS[c]) for h in range(NHG)]

# per-chunk column widths: mid [48 rows] and edge [40 rows], both half-shared
_MW = [len(_chunk_halves[c]) * 6 * C + 2 * CHUNKS[c] * 6 * 128
       for c in range(len(CHUNKS))]
_EW = [len(_chunk_halves[c]) * 2 * C + 2 * CHUNKS[c] * 2 * 128
       for c in range(len(CHUNKS))]
MTOT, ETOT = sum(_MW), sum(_EW)


def _host_prep(features: np.ndarray, masks: np.ndarray):
    """Per-core packed chunk operands fbm [48, MTOT], fbe [40, ETOT] (bf16)."""
    fbms, fbes = [], []
    for i in range(NCORES):
        n, yh = divmod(i, 2)
        feat_pad = np.zeros((HPL, WP, C), np.float32)
        featT = features[n].transpose(1, 2, 0)  # [H, W, C]
        r0 = yh * 32 - 2
        lo, hi = max(0, -r0), min(HPL, H - r0)
        feat_pad[lo:hi, 2:2 + W, :] = featT[r0 + lo:r0 + hi]

        # bnd6[wpw, hpw, hgrp, xblk, pos]
        ml = masks[n, :, yh * 64:(yh + 1) * 64, :]
        bnd6 = np.zeros((12, 8, NHG, NXB, 4, 2, 16), np.float32)
        s = bnd6.strides
        for kr in range(KK):
            for dw in range(KK):
                # dest dims (p4, hgrp, xblk, xw, py, q):
                #   bnd6[dw+xw, p4+kr, hgrp, xblk, p4, py, 2*xw+q]
                dv = np.lib.stride_tricks.as_strided(
                    bnd6[dw, kr],
                    shape=(4, NHG, NXB, 8, 2, 2),
                    strides=(s[1] + s[4], s[2], s[3], s[0] + 2 * s[6], s[5], s[6]),
                )
                sv = ml[kr * KK + dw].reshape(NHG, 4, 2, NXB, 8, 2)
                dv[...] = sv.transpose(1, 0, 3, 4, 2, 5)
        bnd6 = bnd6.reshape(12, 8, NHG, NXB, 128)

        def ftr_half(j, xblk):
            if xblk == 0:
                wps = np.arange(2, 12)      # wp = wpw; wpw 0,1 are pad
            elif xblk == 7:
                wps = np.arange(56, 66)     # wp = 56+wpw; wpw 10,11 are pad
            else:
                wps = 8 * xblk + np.arange(12)
            blk = feat_pad[4 * j:4 * j + 4][:, wps]         # [4 hpr, nw, C]
            return blk.transpose(1, 0, 2).reshape(-1, C)    # k = w*4 + hpr

        def bnd_half(half, hgrp, xblk):
            hp = slice(4 * half, 4 * half + 4)
            if xblk == 0:
                b = bnd6[2:, hp, hgrp, 0]
            elif xblk == 7:
                b = bnd6[:10, hp, hgrp, 7]
            else:
                b = bnd6[:, hp, hgrp, xblk]
            return b.reshape(-1, 128)                       # k = wpw*4 + hpr

        def cat(pieces):
            return np.concatenate(
                [x.transpose(1, 0, 2).reshape(x.shape[1], -1) for x in pieces],
                axis=1)

        fbm_segs, fbe_segs = [], []
        for ci, g in enumerate(CHUNKS):
            h0, js = _h0s[ci], _chunk_halves[ci]
            hs = range(h0, h0 + g)
            if ci == 0:
                # chunk0 mid is split: xblk 1-3 pieces first (small leading
                # DMA lets PE start early), then xblk 4-6
                fbm_segs.append(cat(
                    [np.stack([ftr_half(j, xb) for xb in range(1, 4)]) for j in js]
                    + [np.stack([bnd_half(0, 0, xb) for xb in range(1, 4)])]
                    + [np.stack([bnd_half(1, 0, xb) for xb in range(1, 4)])]
                    + [np.stack([ftr_half(j, xb) for xb in range(4, 7)]) for j in js]
                    + [np.stack([bnd_half(0, 0, xb) for xb in range(4, 7)])]
                    + [np.stack([bnd_half(1, 0, xb) for xb in range(4, 7)])]))
            else:
                fbm_segs.append(cat(
                    [np.stack([ftr_half(j, xb) for xb in range(1, 7)]) for j in js]
                    + [np.stack([bnd_half(0, h, xb) for xb in range(1, 7)]) for h in hs]
                    + [np.stack([bnd_half(1, h, xb) for xb in range(1, 7)]) for h in hs]))
            fbe_segs.append(cat(
                [np.stack([ftr_half(j, xb) for xb in (0, 7)]) for j in js]
                + [np.stack([bnd_half(0, h, xb) for xb in (0, 7)]) for h in hs]
                + [np.stack([bnd_half(1, h, xb) for xb in (0, 7)]) for h in hs]))
        fbms.append(np.concatenate(fbm_segs, axis=1).astype(BF16))
        fbes.append(np.concatenate(fbe_segs, axis=1).astype(BF16))
    return fbms, fbes


_NC_CACHE = []


def _build_nc():
    """Build + compile the single-core Tile program (same for all 8 cores)."""
    if _NC_CACHE:
        return _NC_CACHE[0]

    nc = bacc.Bacc("TRN2", target_bir_lowering=False, debug=False)
    fbm = nc.dram_tensor("fbm", [KH, MTOT], mybir.dt.bfloat16,
                         kind="ExternalInput").ap()
    fbe = nc.dram_tensor("fbe", [KHE, ETOT], mybir.dt.bfloat16,
                         kind="ExternalInput").ap()
    out = nc.dram_tensor("out", [C, 64 * 128], mybir.dt.bfloat16,
                         kind="ExternalOutput").ap()
    ov = out.rearrange("(ch c) (hgrp f) -> ch c hgrp f", ch=2, hgrp=NHG)

    with tile.TileContext(nc) as tc:
        with (
            tc.tile_pool(name="fbp", bufs=2 * len(CHUNKS)) as fbp,
            tc.tile_pool(name="stp", bufs=8) as stp,
            tc.tile_pool(name="pp", bufs=4, space="PSUM") as pp,
        ):
            # warm-up: ACT function table (so it doesn't gate the first real
            # copy) and the PE p-state ramp (~3.4us of dummy matmuls into a
            # scratch psum slice keeps PE continuously busy until real
            # operands arrive, so real matmuls start at full clock)
            dm = stp.tile([KH, 512], mybir.dt.bfloat16, name="dm", tag="dm")
            nc.vector.memset(dm[:], 0)
            nc.scalar.copy(dm[:1, 1:2], dm[:1, 0:1])
            dps = pp.tile([128, 1024], mybir.dt.float32, name="dps", tag="ps")
            for w in range(4):
                nc.tensor.matmul(dps[:, (w % 2) * 512:(w % 2) * 512 + 416],
                                 dm[:, 0:128], dm[:, 0:416], start=True, stop=True)

            mtiles, etiles = [], []
            moff = eoff = 0
            for ci, g in enumerate(CHUNKS):
                tm = fbp.tile([KH, _MW[ci]], mybir.dt.bfloat16,
                              name="fbm", tag="fbm")
                te = fbp.tile([KHE, _EW[ci]], mybir.dt.bfloat16,
                              name="fbe", tag="fbe")
                if ci == 0:
                    # small leading DMA (xblk 1-3 operands) for early PE start
                    nc.sync.dma_start(tm[:, 0:2304], fbm[:, 0:2304])
                    nc.sync.dma_start(tm[:, 2304:4608], fbm[:, moff + 2304:moff + 4608])
                else:
                    nc.sync.dma_start(tm[:], fbm[:, moff:moff + _MW[ci]])
                nc.sync.dma_start(te[:], fbe[:, eoff:eoff + _EW[ci]])
                moff += _MW[ci]
                eoff += _EW[ci]
                mtiles.append(tm)
                etiles.append(te)

            for hgrp in range(NHG):
                ci = _chunk_of_hgrp[hgrp]
                g, nh, hloc = CHUNKS[ci], len(_chunk_halves[ci]), hgrp - _h0s[ci]
                st = stp.tile([128, 2048], mybir.dt.bfloat16, name="st", tag="st")
                dv = st.rearrange(
                    "c (ch p4 py xblk xl) -> c ch xblk p4 py xl",
                    ch=2, p4=4, py=2, xblk=8,
                )
                # two psum half-tiles (2 banks each) per hgrp for finer recycling;
                # edge col-blocks (gated by the fbe DMA) go last in each half
                for hf, xorder in ((0, (1, 2, 3, 0)), (1, (4, 5, 6, 7))):
                    ps = pp.tile([128, 1024], mybir.dt.float32, name="ps", tag="ps")
                    for xblk in xorder:
                        xl4 = xblk - 4 * hf
                        if xblk in (0, 7):
                            # edge col-block: half-shared 40-row matmul pair
                            e = 0 if xblk == 0 else 1
                            for ch in range(2):
                                od = ps[:, (xl4 * 2 + ch) * 128:
                                        (xl4 * 2 + ch + 1) * 128]
                                for half in range(2):
                                    cj, jl = _half_loc[hgrp + half]
                                    lhsT = etiles[cj][
                                        :, (jl * 2 + e) * C + ch * 128:
                                           (jl * 2 + e) * C + ch * 128 + 128]
                                    boff = nh * 2 * C + half * g * 2 * 128
                                    rhs = etiles[ci][
                                        :, boff + (hloc * 2 + e) * 128:
                                           boff + (hloc * 2 + e + 1) * 128]
                                    nc.tensor.matmul(od, lhsT, rhs,
                                                     start=(half == 0),
                                                     stop=(half == 1))
                            continue
                        xb = xblk - 1
                        for ch in range(2):
                            od = ps[:, (xl4 * 2 + ch) * 128:(xl4 * 2 + ch + 1) * 128]
                            for half in range(2):
                                cj, jl = _half_loc[hgrp + half]
                                if cj == 0:
                                    sub, xbs = divmod(xb, 3)
                                    fo = sub * 2304 + jl * 768 + xbs * 256
                                    lhsT = mtiles[0][:, fo + ch * 128:
                                                     fo + ch * 128 + 128]
                                else:
                                    lhsT = mtiles[cj][
                                        :, (jl * 6 + xb) * C + ch * 128:
                                           (jl * 6 + xb) * C + ch * 128 + 128]
                                if ci == 0:
                                    sub, xbs = divmod(xb, 3)
                                    bo = sub * 2304 + 1536 + half * 384 + xbs * 128
                                    rhs = mtiles[0][:, bo:bo + 128]
                                else:
                                    boff = nh * 6 * C + half * g * 6 * 128
                                    rhs = mtiles[ci][
                                        :, boff + (hloc * 6 + xb) * 128:
                                           boff + (hloc * 6 + xb + 1) * 128]
                                nc.tensor.matmul(od, lhsT, rhs,
                                                 start=(half == 0), stop=(half == 1))
                    # psum cols (xblk4, ch, p4, py, xl) -> staging (ch, p4, py, xblk, xl)
                    sv = ps.rearrange(
                        "c (xblk ch p4 py xl) -> c ch xblk p4 py xl",
                        xblk=4, ch=2, p4=4, py=2,
                    )
                    if hgrp == NHG - 1 and hf == 0:
                        # last hgrp: put both hf0 copies on DVE (it has slack)
                        # so hf1 drains on two fresh engines
                        nc.vector.tensor_copy(dv[:, 0, 0:4], sv[:, 0])
                        nc.vector.tensor_copy(dv[:, 1, 0:4], sv[:, 1])
                    else:
                        nc.vector.tensor_copy(dv[:, 0, 4 * hf:4 * hf + 4], sv[:, 0])
                        nc.scalar.copy(dv[:, 1, 4 * hf:4 * hf + 4], sv[:, 1])
                sov = st.rearrange("c (ch f) -> c ch f", ch=2)
                nc.sync.dma_start(ov[:, :, hgrp, :].rearrange("ch c f -> c ch f"), sov)

    nc.compile()
    _NC_CACHE.append(nc)
    return nc


def kernel(features: np.ndarray, masks: np.ndarray) -> np.ndarray:
    features = np.ascontiguousarray(features, dtype=np.float32)
    masks = np.ascontiguousarray(masks, dtype=np.float32)
    fbms, fbes = _host_prep(features, masks)

    nc = _build_nc()
    in_maps = [{"fbm": fbms[i], "fbe": fbes[i]} for i in range(NCORES)]

    res = bass_utils.run_bass_kernel_spmd(nc, in_maps, list(range(NCORES)))

    out = np.empty((N, C, HO, WO), np.float32)
    for i in range(NCORES):
        n, yh = divmod(i, 2)
        out[n, :, yh * 64:(yh + 1) * 64, :] = (
            res.results[i]["out"].astype(np.float32).reshape(C, 64, 128)
        )
    return out
